# revision 26
# baseline (speedup 1.0000x reference)
"""GAT layer on 8 TRN2 cores, row-parallel, fp8-centric, host-projected Wh.

out = elu(softmax_row(mask(adj, lrelu(src_i + dst_j))) @ (h @ W))

Host marshaling: src/dst exact; per-row Schraudolph bias B_i baked into the
adjacency mask bytes (adjB = adj ? B_i : -128); Wh = h @ W computed on host
(the sharding hint replicates Wh) and shipped as e4m3 hi + e4m3 residual lo
with a 16.0 ones column riding along for the softmax denominator.

Device (per core, 1024 query rows):
- scores, in transposed [j, i] layout, int8(max(adj ? lrelu(S(src+dst)) + B_i
  : 0, 0)) which IS the e4m3 bit pattern of exp(lrelu(logit) - C_i)
  (Schraudolph-in-fp8, per-row shifted). Three engine classes balance the
  elementwise work:
    a: one fused custom DVE op per j-tile (lrelu+mask+sat in one pass)
    b: ACT Prelu (lrelu) per j-tile + one DVE STT (mask+sat) per pair
    c: ACT Prelu per j-tile + one Pool (GPSIMD) STT per pair
- aggregation: fp8 DoubleRow matmuls (2 j-tiles per instruction via 3D APs),
  hi (257 cols incl. ones -> denominator) + lo (256) accumulating into 8
  persistent 257-wide PSUM accumulators (one bank per i-tile).
- normalize + elu: reciprocal + elu-combine on DVE, scale on Pool, Exp on ACT.
"""

import numpy as np
import ml_dtypes

import concourse.bass as bass
import concourse.tile as tile
import concourse.mybir as mybir
from concourse import bacc
from concourse.bass_utils import run_bass_kernel_spmd

# ---------------- config ----------------
N_NODES, IN_F, OUT_F = 8192, 512, 256
ALPHA = 0.2
CORES = 8
R = N_NODES // CORES          # rows per core (1024)
RT = R // 128                 # i-tiles per core (8)
JT = N_NODES // 128           # j-tiles (64)
NPT = JT // 2                 # j-tile pairs (32)
SLAB = 4                      # pairs per adj DMA (8KB/partition)
NSLAB = NPT // SLAB           # adj DMAs (8)
WCH = OUT_F + 1               # Wh chunk width incl. ones col (257)
S_BITS = 8.0 / float(np.log(2.0))   # e4m3 bits per nat
ONES_VAL_BITS = 0x58          # e4m3 bit pattern of 16.0
Y_TARGET = 110.0              # per-row max score bits

f32 = mybir.dt.float32
f16 = mybir.dt.float16
bf16 = mybir.dt.bfloat16
i8 = mybir.dt.int8
u8 = mybir.dt.uint8
f8e4 = mybir.dt.float8e4

AT = mybir.AluOpType
AF = mybir.ActivationFunctionType

# pair -> score class: 'a' DVE-fused, 'b' ACT+DVE-STT, 'c' ACT+Pool-STT.
# Counts from the engine-balance LP; interleaved to keep engines co-busy.
N_A, N_B, N_C = 20, 0, 12


def _make_pattern():
    # pairs 0-1 pinned to DVE (earliest-starting engine); last 2 pairs off
    # Pool (latest-finishing engine); the rest largest-remainder interleaved.
    head, tail = ["a", "a"], ["a", "a"]
    counts = {"a": N_A - 4, "b": N_B, "c": N_C}
    n = NPT - len(head) - len(tail)
    acc = {k: 0.0 for k in counts}
    mid = []
    for _ in range(n):
        for k in counts:
            acc[k] += counts[k] / n
        k = max(acc, key=lambda q: acc[q])
        acc[k] -= 1.0
        mid.append(k)
    return head + mid + tail


PAT = _make_pattern()

# ---------------- custom DVE ops ----------------
_REGISTERED = {}


def _get_custom_op():
    if "op" in _REGISTERED:
        return _REGISTERED["op"]
    import concourse.dve_ops as dve_ops
    from concourse.dve_ops import DveOp, _SUB_OPCODE_FOR_NAME
    from concourse.dve_spec import (Spec, Src0, Src1, C0, C1, C2, maxx,
                                    minn, select, Zero, One, lower)
    from concourse.dve_uop import DveOpSpec

    name = "SCHRAU_GAT_ANT"
    _t = Src0 + C0
    spec = Spec(
        body=maxx(select(Src1, maxx(_t, _t * C2) + Src1 + C1, Zero), Zero),
        reference=lambda in0, in1, s0, s1, imm2: np.maximum(
            np.where(
                in1 != 0,
                np.maximum(in0 + s0, (in0 + s0) * imm2) + in1.astype(np.float32) + s1,
                0.0,
            ),
            0.0,
        ).astype(np.float32),
    )
    if name not in _SUB_OPCODE_FOR_NAME:
        row = max(_SUB_OPCODE_FOR_NAME.values()) + 1
        _SUB_OPCODE_FOR_NAME[name] = row
        tmp = DveOpSpec(name=name, opcode=row, uops=lower(spec, ver="v3"), rd1_en=True)
        op = DveOp(name, spec, subdim=False, uops_sha={"v3": tmp.sha("v3")})
        dve_ops.OPS.append(op)
        dve_ops.CUSTOM_DVE_SPECS[name] = spec
    else:
        op = next(o for o in dve_ops.OPS if o.name == name)
    _REGISTERED["op"] = op

    name2 = "ELU_COMBINE_ANT"
    spec2 = Spec(
        body=maxx(Src0, Zero) + minn(Src1 - One, Zero),
        reference=lambda in0, in1, s0, s1, imm2: (
            np.maximum(in0, 0.0) + np.minimum(in1.astype(np.float32) - 1.0, 0.0)
        ).astype(np.float32),
    )
    if name2 not in _SUB_OPCODE_FOR_NAME:
        row2 = max(_SUB_OPCODE_FOR_NAME.values()) + 1
        _SUB_OPCODE_FOR_NAME[name2] = row2
        tmp2 = DveOpSpec(name=name2, opcode=row2, uops=lower(spec2, ver="v3"),
                         rd1_en=True)
        op2 = DveOp(name2, spec2, subdim=False, uops_sha={"v3": tmp2.sha("v3")})
        dve_ops.OPS.append(op2)
        dve_ops.CUSTOM_DVE_SPECS[name2] = spec2
    else:
        op2 = next(o for o in dve_ops.OPS if o.name == name2)
    _REGISTERED["op2"] = op2

    # elu straight from the accumulator: elu = max(acc*rec, 0) + min(qe-1, 0)
    name3 = "ELU_SCALE_ANT"
    spec3 = Spec(
        body=maxx(Src0 * C0, Zero) + minn(Src1 - One, Zero),
        reference=lambda in0, in1, s0, s1, imm2: (
            np.maximum(in0 * s0, 0.0)
            + np.minimum(in1.astype(np.float32) - 1.0, 0.0)
        ).astype(np.float32),
    )
    if name3 not in _SUB_OPCODE_FOR_NAME:
        row3 = max(_SUB_OPCODE_FOR_NAME.values()) + 1
        _SUB_OPCODE_FOR_NAME[name3] = row3
        tmp3 = DveOpSpec(name=name3, opcode=row3, uops=lower(spec3, ver="v3"),
                         rd1_en=True)
        op3 = DveOp(name3, spec3, subdim=False, uops_sha={"v3": tmp3.sha("v3")})
        dve_ops.OPS.append(op3)
        dve_ops.CUSTOM_DVE_SPECS[name3] = spec3
    else:
        op3 = next(o for o in dve_ops.OPS if o.name == name3)
    _REGISTERED["op3"] = op3
    return op


# ---------------- kernel builder ----------------
_BUILD_CACHE = {}


def _build_nc():
    if "nc" in _BUILD_CACHE:
        return _BUILD_CACHE["nc"]
    OP = _get_custom_op()
    OP3 = _REGISTERED["op3"]

    nc = bacc.Bacc("TRN2", target_bir_lowering=False, debug=False,
                   num_devices=CORES)

    # host-packed inputs. hdr = [adj pair0 | adj pair1 | srcb | dstT] so one
    # DMA delivers everything the first score ops need.
    HDR_W = 2 * 2048 + 2 * R + 4 * JT
    hdr_ext = nc.dram_tensor("hdr", [128, HDR_W], i8,
                             kind="ExternalInput").ap()
    # whI: per j-tile chunk = [hi(257) | lo(257)] so hi+lo arrive together
    whI_ext = nc.dram_tensor("whI", [128, JT * 2 * WCH], i8,
                             kind="ExternalInput").ap()
    adjP_ext = nc.dram_tensor("adjP", [NPT * 128, 2048], i8,
                              kind="ExternalInput").ap()
    out_ext = nc.dram_tensor("out", [R, OUT_F], f32, kind="ExternalOutput").ap()

    with tile.TileContext(nc) as tc:
        with tc.tile_pool(name="const", bufs=1) as cpool, \
             tc.tile_pool(name="adj", bufs=4) as apool, \
             tc.tile_pool(name="lrp", bufs=4) as lpool, \
             tc.tile_pool(name="outp", bufs=1) as opool, \
             tc.tile_pool(name="ps", bufs=1, space="PSUM") as pspool:

            # ---- header: adj pairs 0-1 + srcb + dstT in one DMA ----
            hdr = cpool.tile([128, HDR_W], i8, tag="hdr")
            nc.sync.dma_start(out=hdr[:], in_=hdr_ext)
            srcb = hdr[:, 2 * 2048:2 * 2048 + 2 * R].bitcast(f16)
            dstT = hdr[:, 2 * 2048 + 2 * R:].bitcast(f32)
            whI = cpool.tile([128, JT * 2 * WCH], i8, tag="whI")

            # all scores persist in SBUF (64KB/partition) so aggregation
            # order is fully decoupled from score production
            sptall = cpool.tile([128, NPT * 2048], i8, tag="sptall")

            # 8 persistent accumulators, one PSUM bank per i-tile; col 256
            # collects the softmax denominator via the hi ones column.
            accs = [pspool.tile([128, 512], f32, tag=f"b{t}", name=f"acc{t}")
                    for t in range(RT)]

            def do_scores(pt, aslab, k):
                cls = PAT[pt]
                off = k * 2048
                spt = sptall[:, pt * 2048:(pt + 1) * 2048]
                if cls == "a":
                    for half in range(2):
                        jt = 2 * pt + half
                        nc.vector._custom_dve(
                            OP,
                            out=spt[:, half * R:(half + 1) * R],
                            in0=srcb,
                            in1=aslab[:, off + half * R:off + (half + 1) * R],
                            s0=dstT[:, jt:jt + 1],
                            s1=0.0, imm2=ALPHA)
                else:
                    lrp = lpool.tile([128, 2048], f16, tag="lrp", name="lrp")
                    for half in range(2):
                        jt = 2 * pt + half
                        nc.scalar.activation(lrp[:, half * R:(half + 1) * R],
                                             srcb, AF.Prelu,
                                             bias=dstT[:, jt:jt + 1],
                                             alpha=ALPHA)
                    if cls == "b":
                        nc.vector.scalar_tensor_tensor(
                            spt.bitcast(u8), lrp[:], 1.0,
                            aslab[:, off:off + 2048],
                            AT.mult, AT.add)
                    else:
                        # Pool float TT (u8 out is illegal on Pool; f16 out +
                        # an ACT copy does the saturating u8 store)
                        s16 = lpool.tile([128, 2048], f16, tag="s16",
                                         name="s16")
                        nc.gpsimd.tensor_tensor(
                            s16[:], lrp[:],
                            aslab[:, off:off + 2048], AT.add)
                        nc.scalar.activation(spt.bitcast(u8), s16[:], AF.Copy)

            def do_agg(pt):
                sp3 = sptall[:, pt * 2048:(pt + 1) * 2048].bitcast(f8e4) \
                    .rearrange("p (two i) -> p two i", two=2)
                whc = whI[:, pt * 4 * WCH:(pt + 1) * 4 * WCH].bitcast(f8e4) \
                    .rearrange("p (two w) -> p two w", two=2)   # w = 2*WCH
                first = pt == 0
                last = pt == NPT - 1
                for it in range(RT):
                    lhs3 = sp3[:, :, it * 128:(it + 1) * 128]
                    nc.tensor.matmul(
                        accs[it][:, 0:WCH], lhs3, whc[:, :, 0:WCH],
                        start=first, stop=False,
                        perf_mode=mybir.MatmulPerfMode.DoubleRow,
                        skip_group_check=True)
                    nc.tensor.matmul(
                        accs[it][:, 0:OUT_F], lhs3,
                        whc[:, :, WCH:WCH + OUT_F],
                        start=False, stop=last,
                        perf_mode=mybir.MatmulPerfMode.DoubleRow,
                        skip_group_check=True)

            # ---- main loop: stream adj slabs, scores per pair, agg ----
            # DMA_ENGINES is effectively serial, ~85% loaded: small slabs
            # first for a fast start, then wh hi/lo in quarter chunks slotted
            # where the adj stream runs ahead of score consumption. Agg lo
            # matmuls stall in-order on their whLo chunk; PE catches up.
            SLABS = [2, 4, 4, 4, 4, 4, 4, 4]      # pairs 2..31
            QW = 8 * WCH                           # whI chunk bytes (2056):
            NQ = JT * 2 * WCH // QW                # 16 chunks, 2 pairs each

            def wh_chunk(q):
                nc.sync.dma_start(out=whI[:, q * QW:(q + 1) * QW],
                                  in_=whI_ext[:, q * QW:(q + 1) * QW])

            wh_chunk(0)                            # pairs 0-1 (header pairs)
            for pt in (0, 1):                      # adj from the header DMA
                do_scores(pt, hdr, pt)
                do_agg(pt)
            pt = 2
            qn = 1
            for s, ns in enumerate(SLABS):
                aslab = apool.tile([128, 4 * 2048], i8, tag="aslab",
                                   name=f"aslab{s}")
                nc.sync.dma_start(
                    out=aslab[:, 0:ns * 2048].rearrange(
                        "p (k c) -> p k c", k=ns),
                    in_=adjP_ext[pt * 128:(pt + ns) * 128, :]
                    .rearrange("(k p) c -> p k c", k=ns))
                # whI chunks for the pairs this slab covers
                while qn * 2 < pt + ns and qn < NQ:
                    wh_chunk(qn)
                    qn += 1
                for k in range(ns):
                    do_scores(pt, aslab, k)
                    do_agg(pt)
                    pt += 1
            while qn < NQ:
                wh_chunk(qn)
                qn += 1

            # ---- normalize + elu + out (per i-tile, pipelined) ----
            # rec on Pool (pow -1), qe = Exp(acc*rec) on ACT from PSUM, elu
            # in one DVE op from (acc, rec, qe). No intermediate ar tensor.
            for it in range(RT):
                rec = opool.tile([128, 1], f32, tag=f"rec{it}", name="rec")
                nc.vector.reciprocal(rec[:], accs[it][:, OUT_F:OUT_F + 1])
                qe = opool.tile([128, OUT_F], f32, tag=f"qe{it}", name="qe")
                nc.scalar.activation(qe[:], accs[it][:, 0:OUT_F], AF.Exp,
                                     scale=rec[:])
                elu = opool.tile([128, OUT_F], f32, tag=f"elu{it}", name="elu")
                nc.vector._custom_dve(OP3, out=elu[:],
                                      in0=accs[it][:, 0:OUT_F], in1=qe[:],
                                      s0=rec[:], s1=0.0, imm2=0.0)
                nc.sync.dma_start(out=out_ext[it * 128:(it + 1) * 128, :],
                                  in_=elu[:])

    nc.finalize()
    _BUILD_CACHE["nc"] = nc
    return nc


def kernel(h, adj, W, a1, a2):
    h = np.asarray(h, dtype=np.float32)
    W = np.asarray(W, dtype=np.float32)
    a1 = np.asarray(a1, dtype=np.float32)
    a2 = np.asarray(a2, dtype=np.float32)
    adj = np.asarray(adj)

    nc = _build_nc()

    # ---- host marshaling ----
    Wh = h @ W                                               # [N, F] f32
    src = Wh @ a1
    dst = Wh @ a2
    t = src + float(dst.max())
    lr_rowmax = np.maximum(t, t * ALPHA)
    B_i = np.clip(np.round(Y_TARGET - S_BITS * lr_rowmax), 1, 119).astype(np.int8)

    # adjB[i, j] = adj ? B_i : -128; transposed + pair-packed per core:
    # adjP rows pt*128+p cover j-tiles (2pt, 2pt+1), cols [0:1024 | 1024:2048]
    adjB = np.where(adj > 0, B_i[:, None], np.int8(-128)).astype(np.int8)
    adjTB = np.ascontiguousarray(adjB.T)                     # [j, i]

    # Wh as e4m3 hi + residual lo, per j-tile chunk = [hi|ones16|lo|0]
    e4 = ml_dtypes.float8_e4m3fn
    hi = (16.0 * Wh).astype(e4)
    lo = (16.0 * Wh - hi.astype(np.float32)).astype(e4)

    p = np.empty((JT, 128, 2 * WCH), dtype=np.int8)
    p[:, :, :OUT_F] = hi.view(np.int8).reshape(JT, 128, OUT_F)
    p[:, :, OUT_F] = np.int8(ONES_VAL_BITS)
    p[:, :, WCH:WCH + OUT_F] = lo.view(np.int8).reshape(JT, 128, OUT_F)
    p[:, :, WCH + OUT_F] = 0
    whI = np.ascontiguousarray(
        p.transpose(1, 0, 2).reshape(128, JT * 2 * WCH))

    dstT = np.ascontiguousarray(
        (S_BITS * dst).astype(np.float32).reshape(JT, 128).T)  # [128, 64]

    in_maps = []
    for c in range(CORES):
        sl = slice(c * R, (c + 1) * R)
        srcb = np.broadcast_to((S_BITS * src[sl]).astype(np.float16),
                               (128, R))
        slab = adjTB[:, sl]                                   # [8192, 1024]
        adjP = np.ascontiguousarray(
            slab.reshape(NPT, 2, 128, R).transpose(0, 2, 1, 3)
        ).reshape(NPT * 128, 2 * R)
        # hdr = [adj pair0 | adj pair1 | srcb f16 | dstT f32] as bytes
        hdr = np.concatenate([
            adjP[0:128, :], adjP[128:256, :],
            np.ascontiguousarray(srcb).view(np.int8),
            dstT.view(np.int8),
        ], axis=1)
        in_maps.append({
            "hdr": np.ascontiguousarray(hdr),
            "whI": whI,
            "adjP": adjP,
        })
    res = run_bass_kernel_spmd(nc, in_maps, list(range(CORES)))
    out = np.concatenate([res.results[c]["out"] for c in range(CORES)], axis=0)
    return out
